# revision 23
# baseline (speedup 1.0000x reference)
"""Multi-head causal attention (B=4, T=2048, D=1024, H=16, Dh=64) on 8 trn2 cores.

Sharding: 4-way DP over batch x 2-way TP over heads.
Core c handles batch c//2 and heads (c%2)*8 .. (c%2)*8+7.
Each core computes a partial output [T, D] (its heads' contribution through
w_out rows); host sums the two partials per batch.

Per-core device kernel (bf16 matmul operands, fp32 PSUM accumulation):
  v[t, f]   = sum_d xT[d, t] * w_v[d, f]      (v in [tok, feat] layout,
                                               + fused ones column per head)
  qkT[f, t] = sum_d w_qk[d, f] * xT[d, t]     (q/k in [feat, tok] layout)
  attention per (head h, q-block j of 512, k-tile kt of 128):
      S^T[k, q] = sum_d kT[d, k] * qT[d, q]   (two heads of a pair run as
                                               concurrent K=64 row-group MMs)
      P^T = exp(S^T / 8)                      (no max-subtraction: scores ~N(0,1))
      causal mask on diagonal k-tiles via gpsimd affine_select, narrowed to
      the 128-col triangle (cols beyond it are fully live)
      o^T[m, q] = sum_k v_aug[k, m] * P^T[k, q]   (m: 64 v-feats + ones row
                                                   -> row 64 = softmax denominator)
      attn^T[d, q] = o^T[d, q] * recip(o^T[64, q])  (gpsimd partition_broadcast
                                                     of the reciprocal row; no
                                                     PE broadcast matmul)
  y[t, n] = sum_f attn^T[f, t] * w_o[f, n]

Scheduling (v2): the ACT exp costs (2nq+352)/1.2 ns per k-tile period and is
the natural pace-setter; the PE stream is overcommitted.  S^T for period p+1
is emitted at the END of period p's PE work so exp() runs back-to-back on ACT
without waiting behind PV/filler in the in-order PE queue.  All remaining
projection / out-projection matmuls are trickled ONE MATMUL AT A TIME between
the attention matmuls by a deficit-based pump with per-unit deadlines, so the
PE never idles (HAM stays at full clock) and never bursts.  The final q-block's
out-projection is split: head-pairs 0-2 accumulate into SBUF partials during
the last block; only the 8 hp3 matmuls + DVE adds remain in the tail.
"""

import numpy as np
import ml_dtypes

import concourse.mybir as mybir
import concourse.tile as tile
from concourse import bacc, bass_utils

F32 = mybir.dt.float32
BF16 = mybir.dt.bfloat16

D = 1024          # model dim
T = 2048          # tokens per batch
DH = 64           # head dim
NH_LOC = 8        # heads per core
DT = D // 128     # D tiles (contraction)
TT = T // 128     # token tiles
QB = T // 512     # q blocks of 512
VW = DH + 1       # v width incl ones column

# period cost model (ns)
MM_NS = 216.0          # one 128-contraction N=512 projection matmul, back-to-back
ST_FIX = 100.0         # S^T pair fixed overhead beyond stream
PV_FIX = 30.0          # per-PV-matmul fixed overhead

import os
NARROW_MASK = os.environ.get("K_NARROW_MASK", "1") == "1"
GP_BCAST = os.environ.get("K_GP_BCAST", "0") == "1"


def _block_starts():
    """Global period index of the first k-tile period of each (hp, j) block."""
    starts = {}
    p = 0
    for hp in range(4):
        for j in range(QB):
            starts[(hp, j)] = p
            p += 4 * (j + 1)
    return starts, p


def build_kernel():
    nc = bacc.Bacc()
    xT_d = nc.dram_tensor("xT", [D, T], BF16, kind="ExternalInput")
    wqk_d = nc.dram_tensor("w_qk", [D, 1024], BF16, kind="ExternalInput")
    wv_d = nc.dram_tensor("w_v", [D, 512], BF16, kind="ExternalInput")
    wo_d = nc.dram_tensor("w_o", [512, D], BF16, kind="ExternalInput")
    y_d = nc.dram_tensor("y", [T, D], F32, kind="ExternalOutput")

    starts, n_periods = _block_starts()

    with tile.TileContext(nc) as tc:
        with (
            tc.tile_pool(name="big", bufs=1) as big,
            tc.tile_pool(name="ptp", bufs=6) as ptp,
            tc.tile_pool(name="ovp", bufs=8) as ovp,
            tc.tile_pool(name="stg", bufs=2) as stg,
            tc.tile_pool(name="ps_st", bufs=2, space="PSUM") as ps_st,
            tc.tile_pool(name="ps_pv", bufs=2, space="PSUM") as ps_pv,
            tc.tile_pool(name="ps_mm", bufs=2, space="PSUM") as ps_mm,
        ):
            xt = big.tile([128, DT, T], BF16, tag="xt")
            wqk = big.tile([128, DT, 1024], BF16, tag="wqk")
            wv = big.tile([128, DT, 512], BF16, tag="wv")
            wo = big.tile([128, 4, 1024], BF16, tag="wo")
            qk = [big.tile([128, T], BF16, tag=f"qk{i}", name=f"qk{i}") for i in range(8)]
            attn_t = [big.tile([128, T], BF16, tag=f"attn{i}", name=f"attn{i}") for i in range(4)]
            vsb_t = [big.tile([128, 2, NH_LOC * VW], BF16, tag=f"vsb{i}", name=f"vsb{i}") for i in range(8)]
            yp = [big.tile([128, 512], F32, tag=f"yp{i}", name=f"yp{i}") for i in range(8)]
            vsb_r = [t.rearrange("p t (h c) -> p t h c", c=VW) for t in vsb_t]
            ones = big.tile([1, DH], BF16, tag="ones")
            nc.vector.memset(ones, 1.0)

            # ---- input DMAs: one multi-dim descriptor per logical chunk,
            # spread across engine queues so issue doesn't serialize
            xsrc = xT_d[:, :].rearrange("(a p) t -> p a t", a=DT)
            qsrc = wqk_d[:, :].rearrange("(a p) f -> p a f", a=DT)
            vsrc = wv_d[:, :].rearrange("(a p) f -> p a f", a=DT)
            osrc = wo_d[:, :].rearrange("(a p) f -> p a f", a=4)

            nc.gpsimd.dma_start(wv, vsrc)
            nc.sync.dma_start(xt[:, :, 0:512], xsrc[:, :, 0:512])
            nc.scalar.dma_start(wqk[:, :, 0:128], qsrc[:, :, 0:128])
            nc.scalar.dma_start(wqk[:, :, 512:640], qsrc[:, :, 512:640])
            for tb in range(1, QB):
                nc.sync.dma_start(
                    xt[:, :, tb * 512:(tb + 1) * 512],
                    xsrc[:, :, tb * 512:(tb + 1) * 512],
                )
            for f in (1, 5, 2, 6, 3, 7):  # in consumer (head-pair) order
                nc.sync.dma_start(
                    wqk[:, :, f * 128:(f + 1) * 128],
                    qsrc[:, :, f * 128:(f + 1) * 128],
                )
            nc.scalar.dma_start(wo, osrc)

            # ---- filler units: one projection/out group each, emitted one
            # matmul at a time by the pump
            units = []
            period = [0]

            def v_unit(tt):
                st = {}
                def mk(dt):
                    def mm():
                        if dt == 0:
                            st['ps'] = ps_mm.tile([128, 512], F32, tag="mm", name="psmm")
                        nc.tensor.matmul(
                            st['ps'],
                            lhsT=xt[:, dt, tt * 128:(tt + 1) * 128],
                            rhs=wv[:, dt, :],
                            start=(dt == 0),
                            stop=(dt == DT - 1),
                        )
                    return mm
                def fin():
                    nc.vector.tensor_copy(
                        vsb_r[tt // 2][:, tt % 2, :, 0:DH],
                        st['ps'].rearrange("p (h c) -> p h c", c=DH),
                    )
                    nc.vector.memset(vsb_r[tt // 2][:, tt % 2, :, DH], 1.0)
                # PV(kt=tt) of block (0, tt//4) pops at the START of period
                # start+tt+2 (before that period's pump), so the unit must be
                # forced one period earlier; the last two PVs of a block drain
                # AFTER the final period's pump, so nkt-1 suffices there
                j = tt // 4
                dl = starts[(0, j)] + min(tt + 1, 4 * (j + 1) - 1)
                return {'mms': [mk(d) for d in range(DT)], 'fin': fin,
                        'deadline': dl, 'avail': 0, 'cost': MM_NS}

            def qk_unit(f, tb):
                st = {}
                def mk(dt):
                    def mm():
                        if dt == 0:
                            st['ps'] = ps_mm.tile([128, 512], F32, tag="mm", name="psmm")
                        nc.tensor.matmul(
                            st['ps'],
                            lhsT=wqk[:, dt, f * 128:(f + 1) * 128],
                            rhs=xt[:, dt, tb * 512:(tb + 1) * 512],
                            start=(dt == 0),
                            stop=(dt == DT - 1),
                        )
                    return mm
                def fin():
                    nc.vector.tensor_copy(qk[f][:, tb * 512:(tb + 1) * 512], st['ps'])
                p = f % 4
                if f < 4:
                    dl = starts[(p, tb)] - 1
                else:
                    dl = starts[(p, tb)] + 4 * tb - 1
                return {'mms': [mk(d) for d in range(DT)], 'fin': fin,
                        'deadline': dl, 'avail': 0, 'cost': MM_NS}

            def out_unit(tt, nb, hp4s, to_part, avail):
                st = {}
                def mk(i, hp4):
                    def mm():
                        if i == 0:
                            st['ps'] = ps_mm.tile([128, 512], F32, tag="mm", name="psmm")
                        nc.tensor.matmul(
                            st['ps'],
                            lhsT=attn_t[hp4][:, tt * 128:(tt + 1) * 128],
                            rhs=wo[:, hp4, nb * 512:(nb + 1) * 512],
                            start=(i == 0),
                            stop=(i == len(hp4s) - 1),
                        )
                    return mm
                def fin():
                    if to_part:
                        nc.vector.tensor_copy(yp[(tt % 4) * 2 + nb], st['ps'])
                    else:
                        ysb = stg.tile([128, 512], F32, tag="y", bufs=4,
                                       name=f"ysb{tt}_{nb}")
                        nc.vector.tensor_copy(ysb, st['ps'])
                        nc.sync.dma_start(
                            y_d[tt * 128:(tt + 1) * 128, nb * 512:(nb + 1) * 512],
                            ysb,
                        )
                return {'mms': [mk(i, h) for i, h in enumerate(hp4s)], 'fin': fin,
                        'deadline': 10 ** 9, 'avail': avail, 'cost': MM_NS}

            import bisect

            def push_unit(u):
                keys = [x['deadline'] for x in units]
                pos = bisect.bisect_right(keys, u['deadline'])
                if pos == 0 and units and units[0].get('open'):
                    pos = 1  # never displace a partially-emitted head
                units.insert(pos, u)

            def pump(budget_ns, force_before):
                # strict FIFO: at most one partially-emitted unit exists (the
                # head), so at most two ps_mm accumulation groups are ever in
                # flight and the bufs=2 pool rotation stays safe
                spent = 0.0
                while units:
                    u = units[0]
                    forced = u['deadline'] < force_before
                    if not forced and (spent >= budget_ns or u['avail'] > period[0]):
                        break
                    u['mms'].pop(0)()
                    u['open'] = True
                    spent += u['cost']
                    if not u['mms']:
                        u['fin']()
                        units.pop(0)
                return spent

            # seed filler: everything except the upfront work
            for tt in range(1, 16):
                units.append(v_unit(tt))
            for p in range(4):
                for tb in range(QB):
                    if p == 0 and tb == 0:
                        continue
                    units.append(qk_unit(p, tb))
                    units.append(qk_unit(4 + p, tb))
            units.sort(key=lambda u: u['deadline'])

            # ---- upfront: v tile 0, q/k token-block 0 of head pair 0
            for u in (v_unit(0), qk_unit(0, 0), qk_unit(4, 0)):
                for mm in u['mms']:
                    mm()
                u['fin']()

            # ---- attention machinery ----
            def st_pair(hp, j, kt):
                q0 = 128 * (kt - 4 * j) if kt >= 4 * j else 0
                nq = 512 - q0
                stt = ps_st.tile([128, 1024], F32, tag="st")
                nc.tensor.matmul(
                    stt[:, q0:512],
                    lhsT=qk[4 + hp][0:64, kt * 128:(kt + 1) * 128],
                    rhs=qk[hp][0:64, j * 512 + q0:(j + 1) * 512],
                    start=True, stop=True,
                )
                nc.tensor.matmul(
                    stt[:, 512 + q0:1024],
                    lhsT=qk[4 + hp][64:128, kt * 128:(kt + 1) * 128],
                    rhs=qk[hp][64:128, j * 512 + q0:(j + 1) * 512],
                    start=True, stop=True,
                )
                return (stt, hp, j, kt, q0, nq)

            def emit_exp(pend):
                stt, hp, j, kt, q0, nq = pend
                pt = ptp.tile([128, 1024], BF16, tag="pt",
                              name=f"pt{hp}_{j}_{kt}")
                st_r = stt.rearrange("p (h q) -> p h q", h=2)
                pt_r = pt.rearrange("p (h q) -> p h q", h=2)
                nc.scalar.activation(
                    pt_r[:, :, q0:512], st_r[:, :, q0:512],
                    mybir.ActivationFunctionType.Exp, scale=0.125
                )
                if kt >= 4 * j:
                    # causal mask: only the first 128 live cols form a triangle
                    mw = 128 if NARROW_MASK else nq
                    for half in range(2):
                        nc.gpsimd.affine_select(
                            out=pt[:, half * 512 + q0:half * 512 + q0 + mw],
                            in_=pt[:, half * 512 + q0:half * 512 + q0 + mw],
                            compare_op=mybir.AluOpType.is_ge,
                            fill=0.0,
                            base=0,
                            pattern=[[1, mw]],
                            channel_multiplier=-1,
                        )
                return pt

            def push_epilogue(h, j, pvbc):
                ov = ovp.tile([VW, 512], F32, tag="ov", name=f"ov{h}_{j}")
                nc.vector.tensor_copy(ov, pvbc[0:VW, :])
                holder = {}

                def stage1():
                    dn = stg.tile([1, 512], F32, tag="dn", name=f"dn{h}_{j}")
                    rec = stg.tile([1, 512], F32, tag="rec", name=f"rec{h}_{j}")
                    nc.vector.tensor_copy(dn, ov[DH:DH + 1, :])
                    nc.vector.reciprocal_approx_fast(out=rec, in_=dn)
                    if not GP_BCAST:
                        rb16 = stg.tile([1, 512], BF16, tag="rb16",
                                        name=f"rb16{h}_{j}")
                        nc.vector.tensor_copy(rb16, rec)
                        holder['rec'] = rb16
                    else:
                        holder['rec'] = rec

                def stage2():
                    po = (h % 2) * 64
                    if GP_BCAST:
                        rb = stg.tile([DH, 512], F32, tag="rcb", bufs=2,
                                      name=f"rcb{h}_{j}")
                        nc.gpsimd.partition_broadcast(rb, holder['rec'], channels=DH)
                        nc.vector.tensor_mul(
                            attn_t[h // 2][po:po + 64, j * 512:(j + 1) * 512],
                            ov[0:DH, :],
                            rb,
                        )
                    else:
                        bc = ps_mm.tile([128, 512], F32, tag="mm", name="psbc")
                        nc.tensor.matmul(bc[0:DH, :], lhsT=ones,
                                         rhs=holder['rec'], start=True, stop=True)
                        nc.vector.tensor_mul(
                            attn_t[h // 2][po:po + 64, j * 512:(j + 1) * 512],
                            ov[0:DH, :],
                            bc[0:DH, :],
                        )
                p0 = period[0]
                push_unit({'mms': [stage1], 'fin': lambda: None,
                           'deadline': p0 + 1, 'avail': 0, 'cost': 50.0})
                push_unit({'mms': [stage2], 'fin': lambda: None,
                           'deadline': p0 + 2, 'avail': 0, 'cost': 250.0})

            # ---- main attention stream: one k-tile per period; S^T for the
            # NEXT period is emitted at the end of this period's PE work so the
            # exp chain on ACT never waits behind PV/filler in the PE queue.
            blocks = [(hp, j) for hp in range(4) for j in range(QB)]
            pend = st_pair(0, 0, 0)
            vt_pe = [0.0]
            vt_act = [0.0]

            for bi, (hp, j) in enumerate(blocks):
                nkt = 4 * (j + 1)
                pvA = ps_pv.tile([128, 512], F32, tag="pv")
                pvB = ps_pv.tile([128, 512], F32, tag="pv")
                pv_queue = []

                def pv_mms(kt, pt, q0, nq, hp=hp, nkt=nkt, pvA=pvA, pvB=pvB):
                    def go():
                        nc.tensor.matmul(
                            pvA[0:VW, q0:512],
                            lhsT=vsb_r[kt // 2][:, kt % 2, 2 * hp, :],
                            rhs=pt[:, q0:512],
                            start=(kt == 0), stop=(kt == nkt - 1),
                        )
                        nc.tensor.matmul(
                            pvB[0:VW, q0:512],
                            lhsT=vsb_r[kt // 2][:, kt % 2, 2 * hp + 1, :],
                            rhs=pt[:, 512 + q0:1024],
                            start=(kt == 0), stop=(kt == nkt - 1),
                        )
                    return go, 2 * (nq / 2.4 + PV_FIX)

                for kt in range(nkt):
                    base = 0.0
                    if len(pv_queue) >= 2:
                        go, c = pv_queue.pop(0)
                        go()
                        base += c
                    # pace filler against the ACT exp of this period
                    cur_nq = pend[5]
                    target = (2 * cur_nq + 352) / 1.2
                    # next S^T cost
                    if kt + 1 < nkt:
                        nxt = (hp, j, kt + 1)
                    elif bi + 1 < len(blocks):
                        nxt = (blocks[bi + 1][0], blocks[bi + 1][1], 0)
                    else:
                        nxt = None
                    if nxt is not None:
                        nj, nkt_ = nxt[1], nxt[2]
                        nq0 = 128 * (nkt_ - 4 * nj) if nkt_ >= 4 * nj else 0
                        base += (512 - nq0) / 2.4 + ST_FIX
                    budget = max(0.0, vt_act[0] + target - vt_pe[0] - base)
                    rem_p = max(1, n_periods - period[0])
                    rem_mm = sum(len(u['mms']) for u in units)
                    floor = max(0.0, (rem_mm / rem_p - 0.5) * MM_NS)
                    spent = pump(max(budget, floor), period[0] + 1)
                    if nxt is not None:
                        new_pend = st_pair(*nxt)
                    pt = emit_exp(pend)
                    pv_queue.append(pv_mms(pend[3], pt, pend[4], pend[5]))
                    if nxt is not None:
                        pend = new_pend
                    vt_act[0] += target
                    vt_pe[0] += base + spent
                    period[0] += 1

                for go, c in pv_queue:
                    go()
                    vt_pe[0] += c
                push_epilogue(2 * hp, j, pvA)
                push_epilogue(2 * hp + 1, j, pvB)
                if hp == 3:
                    av = period[0] + 3
                    if j < 3:
                        for tt in range(4 * j, 4 * j + 4):
                            for nb in range(2):
                                push_unit(out_unit(tt, nb, (0, 1, 2, 3),
                                                   False, av))
                    else:
                        # hp 0-2 partials for the final q-block were queued at
                        # the end of hp2 (see below); only hp3 remains.
                        pass
                if hp == 2 and j == 3:
                    av = period[0] + 3
                    for tt in range(12, 16):
                        for nb in range(2):
                            push_unit(out_unit(tt, nb, (0, 1, 2), True, av))

            # ---- tail: drain stages and any remaining filler (the yp
            # partials MUST be emitted before the finishers read them), then
            # finish the last q-block's out projection (single hp3 matmul +
            # DVE add of the SBUF partial)
            pump(float('inf'), 10 ** 10)
            for tt in range(12, 16):
                for nb in range(2):
                    ps = ps_mm.tile([128, 512], F32, tag="mm")
                    nc.tensor.matmul(
                        ps,
                        lhsT=attn_t[3][:, tt * 128:(tt + 1) * 128],
                        rhs=wo[:, 3, nb * 512:(nb + 1) * 512],
                        start=True, stop=True,
                    )
                    ysb = stg.tile([128, 512], F32, tag="y", bufs=4,
                                   name=f"ysbf{tt}_{nb}")
                    nc.vector.tensor_add(ysb, ps, yp[(tt % 4) * 2 + nb])
                    nc.sync.dma_start(
                        y_d[tt * 128:(tt + 1) * 128, nb * 512:(nb + 1) * 512],
                        ysb,
                    )


    nc.compile()
    return nc


def _shard_inputs(x, w_qkv, w_out):
    """Build the 8 per-core input maps (matmul operands pre-cast to bf16)."""
    bf16 = ml_dtypes.bfloat16
    in_maps = []
    for c in range(8):
        b = c // 2
        hg = c % 2
        q_cols = slice(hg * 512, hg * 512 + 512)
        k_cols = slice(1024 + hg * 512, 1024 + hg * 512 + 512)
        v_cols = slice(2048 + hg * 512, 2048 + hg * 512 + 512)
        in_maps.append({
            "xT": np.ascontiguousarray(x[b].T).astype(bf16),
            "w_qk": np.ascontiguousarray(
                np.concatenate([w_qkv[:, q_cols], w_qkv[:, k_cols]], axis=1)
            ).astype(bf16),
            "w_v": np.ascontiguousarray(w_qkv[:, v_cols]).astype(bf16),
            "w_o": np.ascontiguousarray(w_out[hg * 512:hg * 512 + 512, :]).astype(bf16),
        })
    return in_maps


def _run(inputs, trace=False):
    x = np.asarray(inputs["x"], dtype=np.float32)
    w_qkv = np.asarray(inputs["w_qkv"], dtype=np.float32)
    w_out = np.asarray(inputs["w_out"], dtype=np.float32)
    nc = build_kernel()
    in_maps = _shard_inputs(x, w_qkv, w_out)
    res = None
    for attempt in range(3):
        try:
            res = bass_utils.run_bass_kernel_spmd(
                nc, in_maps, core_ids=list(range(8)), trace=trace
            )
            break
        except Exception:
            if attempt == 2:
                raise
    assert res is not None
    out = np.empty((4, T, D), dtype=np.float32)
    for b in range(4):
        out[b] = res.results[2 * b]["y"] + res.results[2 * b + 1]["y"]
    return out, res


def kernel(**inputs):
    out, _ = _run(inputs, trace=False)
    return out


# revision 24
# speedup vs baseline: 1.0750x; 1.0750x over previous
"""Multi-head causal attention (B=4, T=2048, D=1024, H=16, Dh=64) on 8 trn2 cores.

Sharding: 4-way DP over batch x 2-way TP over heads.
Core c handles batch c//2 and heads (c%2)*8 .. (c%2)*8+7.
Each core computes a partial output [T, D] (its heads' contribution through
w_out rows); host sums the two partials per batch.

Per-core device kernel (bf16 matmul operands, fp32 PSUM accumulation):
  v[t, f]   = sum_d xT[d, t] * w_v[d, f]      (v in [tok, feat] layout,
                                               + fused ones column per head)
  qkT[f, t] = sum_d w_qk[d, f] * xT[d, t]     (q/k in [feat, tok] layout)
  attention per (head h, q-block j of 512, k-tile kt of 128):
      S^T[k, q] = sum_d kT[d, k] * qT[d, q]   (two heads of a pair run as
                                               concurrent K=64 row-group MMs)
      P^T = exp(S^T / 8)                      (no max-subtraction: scores ~N(0,1))
      causal mask on diagonal k-tiles via gpsimd affine_select, narrowed to
      the 128-col triangle (cols beyond it are fully live)
      o^T[m, q] = sum_k v_aug[k, m] * P^T[k, q]   (m: 64 v-feats + ones row
                                                   -> row 64 = softmax denominator)
      attn^T[d, q] = o^T[d, q] * recip(o^T[64, q])  (gpsimd partition_broadcast
                                                     of the reciprocal row; no
                                                     PE broadcast matmul)
  y[t, n] = sum_f attn^T[f, t] * w_o[f, n]

Scheduling (v2): the ACT exp costs (2nq+352)/1.2 ns per k-tile period and is
the natural pace-setter; the PE stream is overcommitted.  S^T for period p+1
is emitted at the END of period p's PE work so exp() runs back-to-back on ACT
without waiting behind PV/filler in the in-order PE queue.  All remaining
projection / out-projection matmuls are trickled ONE MATMUL AT A TIME between
the attention matmuls by a deficit-based pump with per-unit deadlines, so the
PE never idles (HAM stays at full clock) and never bursts.  The final q-block's
out-projection is split: head-pairs 0-2 accumulate into SBUF partials during
the last block; only the 8 hp3 matmuls + DVE adds remain in the tail.
"""

import numpy as np
import ml_dtypes

import concourse.mybir as mybir
import concourse.tile as tile
from concourse import bacc, bass_utils

F32 = mybir.dt.float32
BF16 = mybir.dt.bfloat16

D = 1024          # model dim
T = 2048          # tokens per batch
DH = 64           # head dim
NH_LOC = 8        # heads per core
DT = D // 128     # D tiles (contraction)
TT = T // 128     # token tiles
QB = T // 512     # q blocks of 512
VW = DH + 1       # v width incl ones column

# period cost model (ns)
MM_NS = 216.0          # one 128-contraction N=512 projection matmul, back-to-back
ST_FIX = 100.0         # S^T pair fixed overhead beyond stream
PV_FIX = 30.0          # per-PV-matmul fixed overhead

import os
NARROW_MASK = os.environ.get("K_NARROW_MASK", "1") == "1"
GP_BCAST = os.environ.get("K_GP_BCAST", "0") == "1"


def _block_starts():
    """Global period index of the first k-tile period of each (hp, j) block."""
    starts = {}
    p = 0
    for hp in range(4):
        for j in range(QB):
            starts[(hp, j)] = p
            p += 4 * (j + 1)
    return starts, p


def build_kernel():
    nc = bacc.Bacc()
    xT_d = nc.dram_tensor("xT", [D, T], BF16, kind="ExternalInput")
    wqk_d = nc.dram_tensor("w_qk", [D, 1024], BF16, kind="ExternalInput")
    wv_d = nc.dram_tensor("w_v", [D, 512], BF16, kind="ExternalInput")
    wo_d = nc.dram_tensor("w_o", [512, D], BF16, kind="ExternalInput")
    y_d = nc.dram_tensor("y", [T, D], F32, kind="ExternalOutput")

    starts, n_periods = _block_starts()

    with tile.TileContext(nc) as tc:
        with (
            tc.tile_pool(name="big", bufs=1) as big,
            tc.tile_pool(name="ptp", bufs=6) as ptp,
            tc.tile_pool(name="ovp", bufs=8) as ovp,
            tc.tile_pool(name="stg", bufs=2) as stg,
            tc.tile_pool(name="ps_st", bufs=2, space="PSUM") as ps_st,
            tc.tile_pool(name="ps_pv", bufs=2, space="PSUM") as ps_pv,
            tc.tile_pool(name="ps_mm", bufs=2, space="PSUM") as ps_mm,
        ):
            xt = big.tile([128, DT, T], BF16, tag="xt")
            wqk = big.tile([128, DT, 1024], BF16, tag="wqk")
            wv = big.tile([128, DT, 512], BF16, tag="wv")
            wo = big.tile([128, 4, 1024], BF16, tag="wo")
            qk = [big.tile([128, T], BF16, tag=f"qk{i}", name=f"qk{i}") for i in range(8)]
            attn_t = [big.tile([128, T], BF16, tag=f"attn{i}", name=f"attn{i}") for i in range(4)]
            vsb_t = [big.tile([128, 2, NH_LOC * VW], BF16, tag=f"vsb{i}", name=f"vsb{i}") for i in range(8)]
            yp = [big.tile([128, 512], F32, tag=f"yp{i}", name=f"yp{i}") for i in range(8)]
            vsb_r = [t.rearrange("p t (h c) -> p t h c", c=VW) for t in vsb_t]
            ones = big.tile([1, DH], BF16, tag="ones")
            nc.vector.memset(ones, 1.0)

            # ---- input DMAs: one multi-dim descriptor per logical chunk,
            # spread across engine queues so issue doesn't serialize
            xsrc = xT_d[:, :].rearrange("(a p) t -> p a t", a=DT)
            qsrc = wqk_d[:, :].rearrange("(a p) f -> p a f", a=DT)
            vsrc = wv_d[:, :].rearrange("(a p) f -> p a f", a=DT)
            osrc = wo_d[:, :].rearrange("(a p) f -> p a f", a=4)

            nc.gpsimd.dma_start(wv[:, 0:4, :], vsrc[:, 0:4, :])
            nc.gpsimd.dma_start(wv[:, 4:8, :], vsrc[:, 4:8, :])
            nc.sync.dma_start(xt[:, 0:4, 0:512], xsrc[:, 0:4, 0:512])
            nc.sync.dma_start(xt[:, 4:8, 0:512], xsrc[:, 4:8, 0:512])
            nc.scalar.dma_start(wqk[:, :, 0:128], qsrc[:, :, 0:128])
            nc.scalar.dma_start(wqk[:, :, 512:640], qsrc[:, :, 512:640])
            for tb in range(1, QB):
                nc.sync.dma_start(
                    xt[:, :, tb * 512:(tb + 1) * 512],
                    xsrc[:, :, tb * 512:(tb + 1) * 512],
                )
            for f in (1, 5, 2, 6, 3, 7):  # in consumer (head-pair) order
                nc.sync.dma_start(
                    wqk[:, :, f * 128:(f + 1) * 128],
                    qsrc[:, :, f * 128:(f + 1) * 128],
                )
            nc.scalar.dma_start(wo, osrc)

            # ---- filler units: one projection/out group each, emitted one
            # matmul at a time by the pump
            units = []
            period = [0]

            def v_unit(tt):
                st = {}
                def mk(dt):
                    def mm():
                        if dt == 0:
                            st['ps'] = ps_mm.tile([128, 512], F32, tag="mm", name="psmm")
                        nc.tensor.matmul(
                            st['ps'],
                            lhsT=xt[:, dt, tt * 128:(tt + 1) * 128],
                            rhs=wv[:, dt, :],
                            start=(dt == 0),
                            stop=(dt == DT - 1),
                        )
                    return mm
                def fin():
                    nc.vector.tensor_copy(
                        vsb_r[tt // 2][:, tt % 2, :, 0:DH],
                        st['ps'].rearrange("p (h c) -> p h c", c=DH),
                    )
                    nc.vector.memset(vsb_r[tt // 2][:, tt % 2, :, DH], 1.0)
                # PV(kt=tt) of block (0, tt//4) pops at the START of period
                # start+tt+2 (before that period's pump), so the unit must be
                # forced one period earlier; the last two PVs of a block drain
                # AFTER the final period's pump, so nkt-1 suffices there
                j = tt // 4
                dl = starts[(0, j)] + min(tt + 1, 4 * (j + 1) - 1)
                return {'mms': [mk(d) for d in range(DT)], 'fin': fin,
                        'deadline': dl, 'avail': 0, 'cost': MM_NS}

            def qk_unit(f, tb):
                st = {}
                def mk(dt):
                    def mm():
                        if dt == 0:
                            st['ps'] = ps_mm.tile([128, 512], F32, tag="mm", name="psmm")
                        nc.tensor.matmul(
                            st['ps'],
                            lhsT=wqk[:, dt, f * 128:(f + 1) * 128],
                            rhs=xt[:, dt, tb * 512:(tb + 1) * 512],
                            start=(dt == 0),
                            stop=(dt == DT - 1),
                        )
                    return mm
                def fin():
                    nc.vector.tensor_copy(qk[f][:, tb * 512:(tb + 1) * 512], st['ps'])
                p = f % 4
                if f < 4:
                    dl = starts[(p, tb)] - 1
                else:
                    dl = starts[(p, tb)] + 4 * tb - 1
                return {'mms': [mk(d) for d in range(DT)], 'fin': fin,
                        'deadline': dl, 'avail': 0, 'cost': MM_NS}

            def out_unit(tt, nb, hp4s, to_part, avail):
                st = {}
                def mk(i, hp4):
                    def mm():
                        if i == 0:
                            st['ps'] = ps_mm.tile([128, 512], F32, tag="mm", name="psmm")
                        nc.tensor.matmul(
                            st['ps'],
                            lhsT=attn_t[hp4][:, tt * 128:(tt + 1) * 128],
                            rhs=wo[:, hp4, nb * 512:(nb + 1) * 512],
                            start=(i == 0),
                            stop=(i == len(hp4s) - 1),
                        )
                    return mm
                def fin():
                    if to_part:
                        nc.vector.tensor_copy(yp[(tt % 4) * 2 + nb], st['ps'])
                    else:
                        ysb = stg.tile([128, 512], F32, tag="y", bufs=4,
                                       name=f"ysb{tt}_{nb}")
                        nc.vector.tensor_copy(ysb, st['ps'])
                        nc.sync.dma_start(
                            y_d[tt * 128:(tt + 1) * 128, nb * 512:(nb + 1) * 512],
                            ysb,
                        )
                return {'mms': [mk(i, h) for i, h in enumerate(hp4s)], 'fin': fin,
                        'deadline': 10 ** 9, 'avail': avail, 'cost': MM_NS}

            import bisect

            def push_unit(u):
                keys = [x['deadline'] for x in units]
                pos = bisect.bisect_right(keys, u['deadline'])
                if pos == 0 and units and units[0].get('open'):
                    pos = 1  # never displace a partially-emitted head
                units.insert(pos, u)

            HORIZON = 24
            fut_mm = [120]  # MMs of out units not yet pushed (24 full + 8x3 partial)

            def pump(n_mms, force_before):
                # strict FIFO: at most one partially-emitted unit exists (the
                # head), so at most two ps_mm accumulation groups are ever in
                # flight and the bufs=2 pool rotation stays safe.  Discretionary
                # pops are limited to units whose deadline is near (or which are
                # avail-gated out units) so far-future work is saved for the
                # late-kernel filler drought; a unit that can no longer finish
                # at 2 mms/period before its deadline is drained preemptively.
                popped = 0
                while units:
                    u = units[0]
                    forced = u['deadline'] < force_before
                    urgent = (u['deadline'] - period[0]) * 2 < len(u['mms'])
                    in_win = (u['deadline'] <= period[0] + HORIZON
                              or u['deadline'] >= 10 ** 8)
                    if u['avail'] > period[0] and not forced:
                        break
                    if not (forced or urgent) and (popped >= n_mms or not in_win):
                        break
                    u['mms'].pop(0)()
                    u['open'] = True
                    if u['cost'] >= 200.0:
                        popped += 1
                    if not u['mms']:
                        u['fin']()
                        units.pop(0)
                return popped

            # seed filler: everything except the upfront work
            for tt in range(1, 16):
                units.append(v_unit(tt))
            for p in range(4):
                for tb in range(QB):
                    if p == 0 and tb == 0:
                        continue
                    units.append(qk_unit(p, tb))
                    units.append(qk_unit(4 + p, tb))
            units.sort(key=lambda u: u['deadline'])

            # ---- upfront: v tile 0, q/k token-block 0 of head pair 0
            for u in (v_unit(0), qk_unit(0, 0), qk_unit(4, 0)):
                for mm in u['mms']:
                    mm()
                u['fin']()

            # ---- attention machinery ----
            def st_pair(hp, j, kt):
                q0 = 128 * (kt - 4 * j) if kt >= 4 * j else 0
                nq = 512 - q0
                stt = ps_st.tile([128, 1024], F32, tag="st")
                nc.tensor.matmul(
                    stt[:, q0:512],
                    lhsT=qk[4 + hp][0:64, kt * 128:(kt + 1) * 128],
                    rhs=qk[hp][0:64, j * 512 + q0:(j + 1) * 512],
                    start=True, stop=True,
                )
                nc.tensor.matmul(
                    stt[:, 512 + q0:1024],
                    lhsT=qk[4 + hp][64:128, kt * 128:(kt + 1) * 128],
                    rhs=qk[hp][64:128, j * 512 + q0:(j + 1) * 512],
                    start=True, stop=True,
                )
                return (stt, hp, j, kt, q0, nq)

            def emit_exp(pend):
                stt, hp, j, kt, q0, nq = pend
                pt = ptp.tile([128, 1024], BF16, tag="pt",
                              name=f"pt{hp}_{j}_{kt}")
                st_r = stt.rearrange("p (h q) -> p h q", h=2)
                pt_r = pt.rearrange("p (h q) -> p h q", h=2)
                nc.scalar.activation(
                    pt_r[:, :, q0:512], st_r[:, :, q0:512],
                    mybir.ActivationFunctionType.Exp, scale=0.125
                )
                if kt >= 4 * j:
                    # causal mask: only the first 128 live cols form a triangle
                    mw = 128 if NARROW_MASK else nq
                    for half in range(2):
                        nc.gpsimd.affine_select(
                            out=pt[:, half * 512 + q0:half * 512 + q0 + mw],
                            in_=pt[:, half * 512 + q0:half * 512 + q0 + mw],
                            compare_op=mybir.AluOpType.is_ge,
                            fill=0.0,
                            base=0,
                            pattern=[[1, mw]],
                            channel_multiplier=-1,
                        )
                return pt

            def push_epilogue(h, j, pvbc):
                ov = ovp.tile([VW, 512], F32, tag="ov", name=f"ov{h}_{j}")
                nc.vector.tensor_copy(ov, pvbc[0:VW, :])
                holder = {}

                def stage1():
                    dn = stg.tile([1, 512], F32, tag="dn", name=f"dn{h}_{j}")
                    rec = stg.tile([1, 512], F32, tag="rec", name=f"rec{h}_{j}")
                    nc.vector.tensor_copy(dn, ov[DH:DH + 1, :])
                    nc.vector.reciprocal_approx_fast(out=rec, in_=dn)
                    if not GP_BCAST:
                        rb16 = stg.tile([1, 512], BF16, tag="rb16",
                                        name=f"rb16{h}_{j}")
                        nc.vector.tensor_copy(rb16, rec)
                        holder['rec'] = rb16
                    else:
                        holder['rec'] = rec

                def stage2():
                    po = (h % 2) * 64
                    if GP_BCAST:
                        rb = stg.tile([DH, 512], F32, tag="rcb", bufs=2,
                                      name=f"rcb{h}_{j}")
                        nc.gpsimd.partition_broadcast(rb, holder['rec'], channels=DH)
                        nc.vector.tensor_mul(
                            attn_t[h // 2][po:po + 64, j * 512:(j + 1) * 512],
                            ov[0:DH, :],
                            rb,
                        )
                    else:
                        bc = ps_mm.tile([128, 512], F32, tag="mm", name="psbc")
                        nc.tensor.matmul(bc[0:DH, :], lhsT=ones,
                                         rhs=holder['rec'], start=True, stop=True)
                        nc.vector.tensor_mul(
                            attn_t[h // 2][po:po + 64, j * 512:(j + 1) * 512],
                            ov[0:DH, :],
                            bc[0:DH, :],
                        )
                p0 = period[0]
                push_unit({'mms': [stage1], 'fin': lambda: None,
                           'deadline': p0 + 1, 'avail': 0, 'cost': 50.0})
                push_unit({'mms': [stage2], 'fin': lambda: None,
                           'deadline': p0 + 2, 'avail': 0, 'cost': 250.0})

            # ---- main attention stream: one k-tile per period; S^T for the
            # NEXT period is emitted at the end of this period's PE work so the
            # exp chain on ACT never waits behind PV/filler in the PE queue.
            blocks = [(hp, j) for hp in range(4) for j in range(QB)]
            pend = st_pair(0, 0, 0)

            for bi, (hp, j) in enumerate(blocks):
                nkt = 4 * (j + 1)
                pvA = ps_pv.tile([128, 512], F32, tag="pv")
                pvB = ps_pv.tile([128, 512], F32, tag="pv")
                pv_queue = []

                def pv_mms(kt, pt, q0, nq, hp=hp, nkt=nkt, pvA=pvA, pvB=pvB):
                    def go():
                        nc.tensor.matmul(
                            pvA[0:VW, q0:512],
                            lhsT=vsb_r[kt // 2][:, kt % 2, 2 * hp, :],
                            rhs=pt[:, q0:512],
                            start=(kt == 0), stop=(kt == nkt - 1),
                        )
                        nc.tensor.matmul(
                            pvB[0:VW, q0:512],
                            lhsT=vsb_r[kt // 2][:, kt % 2, 2 * hp + 1, :],
                            rhs=pt[:, 512 + q0:1024],
                            start=(kt == 0), stop=(kt == nkt - 1),
                        )
                    return go, 2 * (nq / 2.4 + PV_FIX)

                for kt in range(nkt):
                    if len(pv_queue) >= 2:
                        pv_queue.pop(0)[0]()
                    if kt + 1 < nkt:
                        nxt = (hp, j, kt + 1)
                    elif bi + 1 < len(blocks):
                        nxt = (blocks[bi + 1][0], blocks[bi + 1][1], 0)
                    else:
                        nxt = None
                    rem_p = max(1, n_periods - period[0])
                    rem_mm = fut_mm[0] + sum(
                        len(u['mms']) for u in units if u['cost'] >= 200.0)
                    n = max(2, min(4, round(rem_mm / rem_p + 0.25)))
                    pump(n, period[0] + 1)
                    if nxt is not None:
                        new_pend = st_pair(*nxt)
                    pt = emit_exp(pend)
                    pv_queue.append(pv_mms(pend[3], pt, pend[4], pend[5]))
                    if nxt is not None:
                        pend = new_pend
                    period[0] += 1

                for go, c in pv_queue:
                    go()
                push_epilogue(2 * hp, j, pvA)
                push_epilogue(2 * hp + 1, j, pvB)
                if hp == 3:
                    av = period[0] + 3
                    if j < 3:
                        for tt in range(4 * j, 4 * j + 4):
                            for nb in range(2):
                                push_unit(out_unit(tt, nb, (0, 1, 2, 3),
                                                   False, av))
                        fut_mm[0] -= 32
                    else:
                        # hp 0-2 partials for the final q-block were queued at
                        # the end of hp2 (see below); only hp3 remains.
                        pass
                if hp == 2 and j == 3:
                    av = period[0] + 3
                    for tt in range(12, 16):
                        for nb in range(2):
                            push_unit(out_unit(tt, nb, (0, 1, 2), True, av))
                    fut_mm[0] -= 24

            # ---- tail: drain stages and any remaining filler (the yp
            # partials MUST be emitted before the finishers read them), then
            # finish the last q-block's out projection (single hp3 matmul +
            # DVE add of the SBUF partial)
            pump(10 ** 9, 10 ** 10)
            for tt in range(12, 16):
                for nb in range(2):
                    ps = ps_mm.tile([128, 512], F32, tag="mm")
                    nc.tensor.matmul(
                        ps,
                        lhsT=attn_t[3][:, tt * 128:(tt + 1) * 128],
                        rhs=wo[:, 3, nb * 512:(nb + 1) * 512],
                        start=True, stop=True,
                    )
                    ysb = stg.tile([128, 512], F32, tag="y", bufs=4,
                                   name=f"ysbf{tt}_{nb}")
                    nc.vector.tensor_add(ysb, ps, yp[(tt % 4) * 2 + nb])
                    nc.sync.dma_start(
                        y_d[tt * 128:(tt + 1) * 128, nb * 512:(nb + 1) * 512],
                        ysb,
                    )


    nc.compile()
    return nc


def _shard_inputs(x, w_qkv, w_out):
    """Build the 8 per-core input maps (matmul operands pre-cast to bf16)."""
    bf16 = ml_dtypes.bfloat16
    in_maps = []
    for c in range(8):
        b = c // 2
        hg = c % 2
        q_cols = slice(hg * 512, hg * 512 + 512)
        k_cols = slice(1024 + hg * 512, 1024 + hg * 512 + 512)
        v_cols = slice(2048 + hg * 512, 2048 + hg * 512 + 512)
        in_maps.append({
            "xT": np.ascontiguousarray(x[b].T).astype(bf16),
            "w_qk": np.ascontiguousarray(
                np.concatenate([w_qkv[:, q_cols], w_qkv[:, k_cols]], axis=1)
            ).astype(bf16),
            "w_v": np.ascontiguousarray(w_qkv[:, v_cols]).astype(bf16),
            "w_o": np.ascontiguousarray(w_out[hg * 512:hg * 512 + 512, :]).astype(bf16),
        })
    return in_maps


def _run(inputs, trace=False):
    x = np.asarray(inputs["x"], dtype=np.float32)
    w_qkv = np.asarray(inputs["w_qkv"], dtype=np.float32)
    w_out = np.asarray(inputs["w_out"], dtype=np.float32)
    nc = build_kernel()
    in_maps = _shard_inputs(x, w_qkv, w_out)
    res = None
    for attempt in range(3):
        try:
            res = bass_utils.run_bass_kernel_spmd(
                nc, in_maps, core_ids=list(range(8)), trace=trace
            )
            break
        except Exception:
            if attempt == 2:
                raise
    assert res is not None
    out = np.empty((4, T, D), dtype=np.float32)
    for b in range(4):
        out[b] = res.results[2 * b]["y"] + res.results[2 * b + 1]["y"]
    return out, res


def kernel(**inputs):
    out, _ = _run(inputs, trace=False)
    return out
